# revision 1
# baseline (speedup 1.0000x reference)
"""GCN + DiffPool kernel for Trainium2, data-parallel over graphs across 8 NeuronCores.

Model (per graph, n=150 nodes):
  Z1 = relu(An @ (x @ W1) + b1)          An = D^-1/2 (A+I) D^-1/2
  Z2 = relu(An @ (Z1 @ W2) + b2)
  S  = softmax(An @ (Z2 @ Wa) + ba)      [n, 25]
  Zp = S^T @ Z2 ; Ap = S^T @ (A @ S)
  H  = relu(Anp @ (Zp @ Wp) + bp)        pooled GCN, 25 cluster-nodes
  logits = (sum_rows H) @ Wc + bc

Sharding: 64 graphs -> 8 devices x 8 graphs; block-diagonal adjacency means each
device only gets its 8 graphs' 150x150 blocks (shipped with self-loops
pre-added, i.e. A+I) and its node rows of x (feature-major).

Deferred normalization: An @ M = d .* ((A+I) @ (d .* M)) with d = rsqrt(deg+1).
The row factor is folded into the moving operand (m = d.*M, partition-side
scale, cheap); the column factor d[i'] is applied on the PSUM drain of each
An-matmul (free-side scale against a partition-broadcast dT tile), BEFORE the
per-partition bias + relu of the activation. This lets every An-matmul use the
raw shipped (A+I) tiles directly - no normalized-adjacency build, no
[150,1200] elementwise pass, and the only layout shuffle is a tiny [128,16]
DRAM bounce for dT. A @ S is recovered from (A+I) @ S by subtracting S on the
PSUM drain. colsum+partition-broadcast for the pooled column degrees is fused
into the matmul by using a [*,64] ones block as lhsT. No gpsimd ops anywhere
(SWDGE DMAs and custom-op lib load/unload are expensive).

On-device layout convention:
  fm (feature-major): [feat_part, graph, node]  - used for W-multiplies (lhsT)
  nm (node-major):    [node_part, graph, feat]  - used for A-multiplies
Node dim (150) splits into partition chunks c0=[0:128], c1=[128:150].
"""

import numpy as np

import concourse.bass as bass
import concourse.mybir as mybir
import concourse.tile as tile
from concourse import bacc
from concourse.bass_utils import run_bass_kernel_spmd

F32 = mybir.dt.float32
BF16 = mybir.dt.bfloat16
AF = mybir.ActivationFunctionType
AL = mybir.AluOpType
U32 = mybir.dt.uint32

MMDT = BF16

N_NODES = 9600
N_FEAT = 128
HIDDEN = 64
CLUSTERS = 25
NUM_CLASSES = 10
B_GRAPHS = 64
NPG = 150            # nodes per graph
DEV = 8              # devices
GPD = 8              # graphs per device
C0, C1 = 128, 22     # node partition chunks (128 + 22 = 150)

_CACHE = {}

# wpk (bf16) packed-constant column offsets
WP_W1 = 0                      # [128, 64]
WP_W2 = WP_W1 + HIDDEN         # [64, 64]
WP_WA = WP_W2 + HIDDEN         # [64, 25]
WP_WP = WP_WA + CLUSTERS       # [64, 64]
WP_ONES = WP_WP + HIDDEN       # [128, 128] all-ones block (colsum/bcast lhsT)
WP_ID64 = WP_ONES + N_FEAT     # [64, 64] identity (z2 transposes)
WP_ID128 = WP_ID64 + HIDDEN    # [128, 128] identity (d transposes)
WP_COLS = WP_ID128 + N_FEAT

# fpk (f32) packed-constant column offsets
FP_WC = 0                      # [64, 10]
FP_B1 = FP_WC + NUM_CLASSES    # [64, 1]
FP_B2 = FP_B1 + 1
FP_BP = FP_B2 + 1
FP_BC = FP_BP + 1              # [8, 10] bc broadcast over graphs
FP_ID25 = FP_BC + NUM_CLASSES  # [25, 25] identity
FP_BA = FP_ID25 + CLUSTERS     # [128, 25] ba broadcast over partitions
FP_COLS = FP_BA + CLUSTERS


def _chunk(c):
    return (0, C0) if c == 0 else (C0, C1)


def build_nc():
    nc = bacc.Bacc("TRN2", target_bir_lowering=False, debug=False, num_devices=DEV)

    def din(name, shape, dt=F32):
        return nc.dram_tensor(name, shape, dt, kind="ExternalInput").ap()

    ah0 = din("ah0", [C0, GPD, NPG], MMDT)   # (A+I) rows 0:128 per graph
    ah1 = din("ah1", [C1, GPD, NPG], MMDT)   # (A+I) rows 128:150
    xT = din("xT", [N_FEAT, GPD, NPG], MMDT)
    wpk = din("wpk", [N_FEAT, WP_COLS], MMDT)
    fpk = din("fpk", [N_FEAT, FP_COLS], F32)
    selp = din("selp", [GPD, GPD * N_FEAT], MMDT)  # one-hot row selectors
    out = nc.dram_tensor("out", [GPD, NUM_CLASSES], F32, kind="ExternalOutput").ap()

    with tile.TileContext(nc) as tc:
        with (
            tc.tile_pool(name="cst", bufs=1) as cst,
            tc.tile_pool(name="act", bufs=1) as act,
            tc.tile_pool(name="ps", bufs=7, space="PSUM") as ps,
            tc.tile_pool(name="pst", bufs=1, space="PSUM") as pst,
            tc.tile_pool(name="dram", bufs=1, space="DRAM") as dram,
        ):
            # ---- input DMAs, all HWDGE (sync/scalar). Adjacency first: it
            # heads the degree->rsqrt->An critical chain. -------------------
            HG = GPD // 2
            s_ah0 = cst.tile([C0, GPD, NPG], MMDT, tag="ah0")
            nc.sync.dma_start(out=s_ah0[:, 0:HG, :], in_=ah0[:, 0:HG, :])
            s_ah1 = cst.tile([C1, GPD, NPG], MMDT, tag="ah1")
            nc.scalar.dma_start(out=s_ah1[:], in_=ah1)
            nc.sync.dma_start(out=s_ah0[:, HG:GPD, :], in_=ah0[:, HG:GPD, :])
            s_wpk = cst.tile([N_FEAT, WP_COLS], MMDT, tag="wpk")
            nc.scalar.dma_start(out=s_wpk[:], in_=wpk)
            s_xT = cst.tile([N_FEAT, GPD, NPG], MMDT, tag="xT")
            nc.sync.dma_start(out=s_xT[:], in_=xT)
            s_fpk = cst.tile([N_FEAT, FP_COLS], F32, tag="fpk")
            nc.scalar.dma_start(out=s_fpk[:], in_=fpk)
            s_selp = cst.tile([GPD, GPD * N_FEAT], MMDT, tag="selp")
            nc.scalar.dma_start(out=s_selp[:], in_=selp)

            s_W1 = s_wpk[:, WP_W1:WP_W1 + HIDDEN]
            s_W2 = s_wpk[0:HIDDEN, WP_W2:WP_W2 + HIDDEN]
            s_Wa = s_wpk[0:HIDDEN, WP_WA:WP_WA + CLUSTERS]
            s_Wp = s_wpk[0:HIDDEN, WP_WP:WP_WP + HIDDEN]
            s_ones = s_wpk[:, WP_ONES:WP_ONES + HIDDEN]
            s_id64 = s_wpk[0:HIDDEN, WP_ID64:WP_ID64 + HIDDEN]
            s_id128 = s_wpk[:, WP_ID128:WP_ID128 + N_FEAT]
            s_Wc = s_fpk[0:HIDDEN, FP_WC:FP_WC + NUM_CLASSES]
            s_b1 = s_fpk[0:HIDDEN, FP_B1:FP_B1 + 1]
            s_b2 = s_fpk[0:HIDDEN, FP_B2:FP_B2 + 1]
            s_bp = s_fpk[0:HIDDEN, FP_BP:FP_BP + 1]
            s_bc = s_fpk[0:GPD, FP_BC:FP_BC + NUM_CLASSES]
            s_id25 = s_fpk[0:CLUSTERS, FP_ID25:FP_ID25 + CLUSTERS]
            s_baB = s_fpk[:, FP_BA:FP_BA + CLUSTERS]

            # ---- rsqrt helper (quake seed + Newton), all on DVE ------------
            qk1 = act.tile([C0, 1], U32, tag="qk1")
            nc.vector.memset(qk1[:], 1)
            qkm = act.tile([C0, 1], U32, tag="qkm")
            nc.vector.memset(qkm[:], 0x5F3759DF)

            def emit_rsqrt(x, rows, cols, iters=1):
                s = act.tile([rows, cols], F32, tag=f"rs_{id(x)}")
                w = act.tile([rows, cols], F32, tag=f"rw_{id(x)}")
                nc.vector.tensor_tensor(s[:].bitcast(U32), x[:].bitcast(U32),
                                        qk1[0:rows, :].broadcast_to((rows, cols)),
                                        AL.logical_shift_right)
                nc.vector.tensor_tensor(s[:].bitcast(U32),
                                        qkm[0:rows, :].broadcast_to((rows, cols)),
                                        s[:].bitcast(U32), AL.subtract)
                for _ in range(iters):
                    nc.vector.tensor_mul(w[:], s[:], s[:])
                    nc.vector.tensor_mul(w[:], w[:], x[:])
                    nc.vector.tensor_scalar(w[:], w[:], -0.5, 1.5, AL.mult, AL.add)
                    nc.vector.tensor_mul(s[:], s[:], w[:])
                return s

            # ---- d = rsqrt(rowsum(A+I)) in node-major [chunk, graph].
            # a1 reduce first (its DMA lands first); a0 in halves. ----------
            degc = act.tile([C0, 2 * GPD], F32, tag="degc")
            nc.vector.memset(degc[:, GPD:2 * GPD], 1.0)  # unread rows stay finite
            nc.vector.reduce_sum(out=degc[:, 0:HG], in_=s_ah0[:, 0:HG, :],
                                 axis=mybir.AxisListType.X)
            nc.vector.reduce_sum(out=degc[:, HG:GPD], in_=s_ah0[:, HG:GPD, :],
                                 axis=mybir.AxisListType.X)
            nc.vector.reduce_sum(out=degc[0:C1, GPD:2 * GPD], in_=s_ah1[:],
                                 axis=mybir.AxisListType.X)
            dcomb = emit_rsqrt(degc, C0, 2 * GPD, iters=1)
            s_d = [dcomb[:, 0:GPD], dcomb[0:C1, GPD:2 * GPD]]
            dbfc = act.tile([C0, 2 * GPD], MMDT, tag="dbfc")
            nc.vector.tensor_copy(dbfc[:], dcomb[:])

            # ---- dT broadcast fully on-chip: PE-transpose d to [8,150],
            # then per-graph K=1 ones-outer-product matmuls replicate row g
            # across all 128 partitions into PSUM (no DMA in this chain) ----
            p_dt = pst.tile([GPD, NPG], MMDT, tag="p2")
            nc.tensor.transpose(p_dt[:, 0:C0], dbfc[:, 0:GPD], s_id128)
            nc.tensor.transpose(p_dt[:, C0:NPG], dbfc[0:C1, GPD:2 * GPD],
                                s_id128[0:C1, 0:C1])
            dTrow = act.tile([GPD, NPG], MMDT, tag="dTrow")
            nc.vector.tensor_copy(dTrow[:], p_dt[:])

            # ---- helpers ---------------------------------------------------
            def w_mult_nm(lhs_fm, w, kdim, fout, name, ba_row=False):
                """m = d .* (Z @ W), node-major chunks. lhsT = fm slice.
                ba_row: chunk-1 tile gets an extra row 22 holding ba (pairs
                with the an1 ones row)."""
                outs = []
                for c, cn in ((0, C0), (1, C1)):
                    off, _ = _chunk(c)
                    rows = cn + (1 if (c == 1 and ba_row) else 0)
                    p = ps.tile([cn, GPD, fout], F32, tag="ps")
                    for g in range(GPD):
                        nc.tensor.matmul(p[:, g, :], lhs_fm[0:kdim, g, off:off + cn],
                                         w, start=True, stop=True)
                    o = act.tile([rows, GPD, fout], MMDT, tag=f"{name}{c}")
                    if c == 1 and ba_row:
                        # ba into every row first; rows 0:22 overwritten below
                        bsrc = s_baB[0:rows, 0:fout][:, None, :] \
                            .broadcast_to((rows, GPD, fout))
                        nc.vector.tensor_copy(o[:], bsrc)
                    dbc = s_d[c][:][:, :, None].broadcast_to((cn, GPD, fout))
                    nc.vector.tensor_mul(o[0:cn], p[:], dbc)
                    outs.append(o)
                return outs


            # m1 = d .* (x @ W1) BEFORE the An build so its DVE drain runs
            # while the dT bounce DMAs are in flight.
            m1 = w_mult_nm(s_xT, s_W1, N_FEAT, HIDDEN, "m1")

            # ---- An_col = (A+I) .* dT_bc per graph pair.  an1 carries an
            # all-ones contraction row 22 (pairs with ba row in v for the
            # softmax bias fold). -------------------------------------------
            an0 = act.tile([C0, GPD, NPG], MMDT, tag="an0")
            an1 = act.tile([C1 + 1, GPD, NPG], MMDT, tag="an1")
            # ones in all 23 rows; rows 0:22 are overwritten by the An muls
            # below (partition-22-only writes are not DVE-addressable)
            nc.vector.memset(an1[:], 1.0)
            for q in range(GPD // 2):
                g = 2 * q
                dtb = ps.tile([C0, 2, NPG], F32, tag="ps")
                nc.tensor.matmul(dtb[:, 0, :],
                                 s_selp[:, g * N_FEAT:(g + 1) * N_FEAT],
                                 dTrow[:], start=True, stop=True)
                nc.tensor.matmul(dtb[:, 1, :],
                                 s_selp[:, (g + 1) * N_FEAT:(g + 2) * N_FEAT],
                                 dTrow[:], start=True, stop=True)
                nc.vector.tensor_mul(an0[:, g:g + 2, :], s_ah0[:, g:g + 2, :],
                                     dtb[:])
                nc.vector.tensor_mul(an1[0:C1, g:g + 2, :], s_ah1[:, g:g + 2, :],
                                     dtb[0:C1, :, :])

            def an_mult_fm(m_nm, bias, name):
                """fm out [64, g, 150] = relu(An @ m + bias), graph-paired
                PSUM tiles, ACT-only drain."""
                o = act.tile([HIDDEN, GPD, NPG], MMDT, tag=name)
                for q in range(GPD // 2):
                    p = ps.tile([HIDDEN, 2, NPG], F32, tag="ps")
                    for k in range(2):
                        g = 2 * q + k
                        nc.tensor.matmul(p[:, k, :], m_nm[0][:, g, :],
                                         an0[:, g, :], start=True, stop=False)
                        nc.tensor.matmul(p[:, k, :], m_nm[1][:, g, :],
                                         an1[0:C1, g, :], start=False, stop=True)
                    nc.scalar.activation(o[:, 2 * q:2 * q + 2, :], p[:],
                                         AF.Relu, bias=bias)
                return o

            # ---- encoder ---------------------------------------------------
            z1 = an_mult_fm(m1, s_b1, "z1")
            m2 = w_mult_nm(z1, s_W2, HIDDEN, HIDDEN, "m2")
            z2 = an_mult_fm(m2, s_b2, "z2")

            # ---- Z2 transpose -> nm (for pooling contractions) -------------
            z2n = []
            for c, cn in ((0, C0), (1, C1)):
                off, _ = _chunk(c)
                p = pst.tile([cn, GPD, HIDDEN], MMDT, tag="p2")
                for g in range(GPD):
                    nc.tensor.transpose(p[:, g, :], z2[0:HIDDEN, g, off:off + cn],
                                        s_id64)
                o = act.tile([cn, GPD, HIDDEN], MMDT, tag=f"z2n{c}")
                nc.scalar.copy(o[:], p[:])
                z2n.append(o)

            # ---- assignment: S = softmax(An @ v + ba), nm.  ba rides the
            # an1 ones-row x v1 ba-row rank-1 term inside the matmul. -------
            v = w_mult_nm(z2, s_Wa, HIDDEN, CLUSTERS, "v", ba_row=True)
            s_P, s_S = [], []
            for mc, mn in ((0, C0), (1, C1)):
                moff, _ = _chunk(mc)
                p = ps.tile([mn, GPD, CLUSTERS], F32, tag="ps")
                for g in range(GPD):
                    nc.tensor.matmul(p[:, g, :], an0[:, g, moff:moff + mn],
                                     v[0][:, g, :], start=True, stop=False)
                    nc.tensor.matmul(p[:, g, :], an1[0:C1 + 1, g, moff:moff + mn],
                                     v[1][0:C1 + 1, g, :], start=False, stop=True)
                s_P.append(p)
                st = act.tile([mn, GPD, CLUSTERS], MMDT, tag=f"s{mc}")
                s_S.append(st)
            # softmax drains by graph half (c0h0, c1h0 first) so the AS
            # matmuls for the first graphs start one half-chain earlier
            for h in range(2):
                gl, gh = h * HG, (h + 1) * HG
                for mc, mn in ((0, C0), (1, C1)):
                    p, s = s_P[mc], s_S[mc]
                    e = act.tile([mn, HG, CLUSTERS], F32, tag=f"e{mc}{h}")
                    nc.scalar.activation(e[:], p[:, gl:gh, :], AF.Exp)
                    ssum = act.tile([mn, HG], F32, tag=f"ssum{mc}{h}")
                    nc.vector.reduce_sum(out=ssum[:], in_=e[:],
                                         axis=mybir.AxisListType.X)
                    rs = act.tile([mn, HG], F32, tag=f"rs{mc}{h}")
                    nc.vector.reciprocal(rs[:], ssum[:])
                    nc.vector.tensor_mul(
                        s[:, gl:gh, :], e[:],
                        rs[:][:, :, None].broadcast_to((mn, HG, CLUSTERS)))

            # ---- AS = A @ S = (A+I) @ S - S, nm ----------------------------
            s_AS = []
            for mc, mn in ((0, C0), (1, C1)):
                moff, _ = _chunk(mc)
                p = ps.tile([mn, GPD, CLUSTERS], F32, tag="ps")
                for g in range(GPD):
                    nc.tensor.matmul(p[:, g, :], s_ah0[:, g, moff:moff + mn],
                                     s_S[0][:, g, :], start=True, stop=False)
                    nc.tensor.matmul(p[:, g, :], s_ah1[:, g, moff:moff + mn],
                                     s_S[1][:, g, :], start=False, stop=True)
                o = act.tile([mn, GPD, CLUSTERS], MMDT, tag=f"as{mc}")
                for h in range(2):
                    gl, gh = h * HG, (h + 1) * HG
                    nc.vector.tensor_tensor(o[:, gl:gh, :], p[:, gl:gh, :],
                                            s_S[mc][:, gl:gh, :], AL.subtract)
                s_AS.append(o)

            # ---- Ap = S^T @ AS (PSUM), row degrees + dp first (dp gates the
            # H-matmul chain; the dpT chain below only gates the H drain) ----
            p_ap = ps.tile([CLUSTERS, GPD, CLUSTERS], F32, tag="ps")
            for g in range(GPD):
                nc.tensor.matmul(p_ap[:, g, :], s_S[0][:, g, :], s_AS[0][:, g, :],
                                 start=True, stop=False)
                nc.tensor.matmul(p_ap[:, g, :], s_S[1][:, g, :], s_AS[1][:, g, :],
                                 start=False, stop=True)
            degp = act.tile([CLUSTERS, GPD], F32, tag="degp")
            nc.vector.reduce_sum(out=degp[:, 0:HG], in_=p_ap[:, 0:HG, :],
                                 axis=mybir.AxisListType.X)
            nc.vector.reduce_sum(out=degp[:, HG:GPD], in_=p_ap[:, HG:GPD, :],
                                 axis=mybir.AxisListType.X)
            nc.vector.tensor_scalar_add(degp[:], degp[:], 1.0)
            dpw = act.tile([CLUSTERS, GPD], F32, tag="dpw")
            nc.vector.tensor_tensor(dpw[:].bitcast(U32), degp[:].bitcast(U32),
                                    qk1[0:CLUSTERS, :].broadcast_to((CLUSTERS, GPD)),
                                    AL.logical_shift_right)
            nc.vector.tensor_tensor(dpw[:].bitcast(U32),
                                    qkm[0:CLUSTERS, :].broadcast_to((CLUSTERS, GPD)),
                                    dpw[:].bitcast(U32), AL.subtract)
            dpw2 = act.tile([CLUSTERS, GPD], F32, tag="dpw2")
            nc.vector.tensor_mul(dpw2[:], dpw[:], dpw[:])
            nc.vector.tensor_mul(dpw2[:], dpw2[:], degp[:])
            nc.vector.tensor_scalar(dpw2[:], dpw2[:], -0.5, 1.5, AL.mult, AL.add)
            dp = act.tile([CLUSTERS, GPD], MMDT, tag="dp")
            nc.vector.tensor_mul(dp[:], dpw[:], dpw2[:])

            # ---- Zp = S^T @ Z2, fm [64, g, 25] (before the dpT PE ops: those
            # wait on the DVE rsqrt chain and would block the PE queue) ------
            p_zp = ps.tile([HIDDEN, GPD, CLUSTERS], F32, tag="ps")
            for g in range(GPD):
                nc.tensor.matmul(p_zp[:, g, :], z2n[0][:, g, :], s_S[0][:, g, :],
                                 start=True, stop=False)
                nc.tensor.matmul(p_zp[:, g, :], z2n[1][:, g, :], s_S[1][:, g, :],
                                 start=False, stop=True)
            s_Zp = act.tile([HIDDEN, GPD, CLUSTERS], MMDT, tag="zp")
            nc.scalar.copy(s_Zp[:], p_zp[:])

            # ---- dpT [64, g, 25] = dp transposed + partition-broadcast (Ap
            # is symmetric so row degrees == column degrees): cast, PE
            # transpose [25,8]->[8,25], then selector matmuls ---------------
            p_dpt = pst.tile([GPD, CLUSTERS], MMDT, tag="p2")
            nc.tensor.transpose(p_dpt[:], dp[:], s_id128[0:CLUSTERS, 0:CLUSTERS])
            dpTrow = act.tile([GPD, CLUSTERS], MMDT, tag="dpTrow")
            nc.vector.tensor_copy(dpTrow[:], p_dpt[:])
            p_dpb = ps.tile([HIDDEN, GPD, CLUSTERS], F32, tag="ps")
            for g in range(GPD):
                nc.tensor.matmul(p_dpb[:, g, :],
                                 s_selp[:, g * N_FEAT:g * N_FEAT + HIDDEN],
                                 dpTrow[:], start=True, stop=True)
            s_dpT = act.tile([HIDDEN, GPD, CLUSTERS], F32, tag="dpt")
            nc.scalar.copy(s_dpT[:], p_dpb[:])

            # ---- ahp = Ap + I (raw, normalization deferred) ----------------
            ahp = act.tile([CLUSTERS, GPD, CLUSTERS], MMDT, tag="ahp")
            id25b = s_id25[:, None, :].broadcast_to((CLUSTERS, GPD, CLUSTERS))
            nc.vector.tensor_add(ahp[:], p_ap[:], id25b)

            # ---- pooled GCN: H = relu(dp' .* ((Ap+I) @ (dp .* ZpWp)) + bp) -
            p_zw = ps.tile([CLUSTERS, GPD, HIDDEN], F32, tag="ps")
            for g in range(GPD):
                nc.tensor.matmul(p_zw[:, g, :], s_Zp[:, g, :], s_Wp,
                                 start=True, stop=True)
            mp = act.tile([CLUSTERS, GPD, HIDDEN], MMDT, tag="mp")
            nc.vector.tensor_mul(mp[:], p_zw[:],
                                 dp[:][:, :, None].broadcast_to((CLUSTERS, GPD, HIDDEN)))

            p_h = ps.tile([HIDDEN, GPD, CLUSTERS], F32, tag="ps")
            for g in range(GPD):
                nc.tensor.matmul(p_h[:, g, :], mp[:, g, :], ahp[:, g, :],
                                 start=True, stop=True)
            th = act.tile([HIDDEN, GPD, CLUSTERS], F32, tag="th")
            s_H = act.tile([HIDDEN, GPD, CLUSTERS], F32, tag="h")
            s_G = act.tile([HIDDEN, GPD], F32, tag="g")
            for h in range(2):
                gl, gh = h * HG, (h + 1) * HG
                nc.vector.tensor_mul(th[:, gl:gh, :], p_h[:, gl:gh, :],
                                     s_dpT[:, gl:gh, :])
                nc.scalar.activation(s_H[:, gl:gh, :], th[:, gl:gh, :],
                                     AF.Relu, bias=s_bp)
                nc.vector.reduce_sum(out=s_G[:, gl:gh], in_=s_H[:, gl:gh, :],
                                     axis=mybir.AxisListType.X)

            p_l = ps.tile([GPD, NUM_CLASSES], F32, tag="ps")
            nc.tensor.matmul(p_l[:], s_G[:], s_Wc, start=True, stop=True)
            s_out = act.tile([GPD, NUM_CLASSES], F32, tag="logits")
            nc.vector.tensor_add(s_out[:], p_l[:], s_bc)
            nc.sync.dma_start(out=out, in_=s_out[:])

    nc.compile()
    return nc


def make_in_maps(x, a, W1, b1, W2, b2, Wa, ba, Wp, bp, Wc, bc):
    import ml_dtypes
    npmm = np.dtype(ml_dtypes.bfloat16) if MMDT == BF16 else np.dtype(np.float32)

    x = np.ascontiguousarray(np.asarray(x, dtype=np.float32))
    a = np.asarray(a, dtype=np.float32)

    # diagonal 150x150 blocks of the batch adjacency, self-loops pre-added
    ab = a.reshape(B_GRAPHS, NPG, B_GRAPHS, NPG)
    blocks = ab[np.arange(B_GRAPHS), :, np.arange(B_GRAPHS), :]  # [64, 150, 150]
    blocks = blocks + np.eye(NPG, dtype=np.float32)[None]
    blocks = blocks.astype(npmm)

    wpk = np.zeros((N_FEAT, WP_COLS), npmm)
    wpk[:, WP_W1:WP_W1 + HIDDEN] = np.asarray(W1, np.float32).astype(npmm)
    wpk[0:HIDDEN, WP_W2:WP_W2 + HIDDEN] = np.asarray(W2, np.float32).astype(npmm)
    wpk[0:HIDDEN, WP_WA:WP_WA + CLUSTERS] = np.asarray(Wa, np.float32).astype(npmm)
    wpk[0:HIDDEN, WP_WP:WP_WP + HIDDEN] = np.asarray(Wp, np.float32).astype(npmm)
    wpk[:, WP_ONES:WP_ONES + N_FEAT] = 1.0
    wpk[0:HIDDEN, WP_ID64:WP_ID64 + HIDDEN] = np.eye(HIDDEN, dtype=npmm)
    wpk[:, WP_ID128:WP_ID128 + N_FEAT] = np.eye(N_FEAT, dtype=npmm)

    fpk = np.zeros((N_FEAT, FP_COLS), np.float32)
    fpk[0:HIDDEN, FP_WC:FP_WC + NUM_CLASSES] = np.asarray(Wc, np.float32)
    fpk[0:HIDDEN, FP_B1] = np.asarray(b1, np.float32)
    fpk[0:HIDDEN, FP_B2] = np.asarray(b2, np.float32)
    fpk[0:HIDDEN, FP_BP] = np.asarray(bp, np.float32)
    fpk[0:GPD, FP_BC:FP_BC + NUM_CLASSES] = np.asarray(bc, np.float32)[None, :]
    fpk[0:CLUSTERS, FP_ID25:FP_ID25 + CLUSTERS] = np.eye(CLUSTERS, dtype=np.float32)
    fpk[:, FP_BA:FP_BA + CLUSTERS] = np.asarray(ba, np.float32)[None, :]

    selp = np.zeros((GPD, GPD * N_FEAT), npmm)
    for g in range(GPD):
        selp[g, g * N_FEAT:(g + 1) * N_FEAT] = 1.0

    common = dict(wpk=wpk, fpk=fpk, selp=selp)

    in_maps = []
    for d in range(DEV):
        xd = x[d * GPD * NPG:(d + 1) * GPD * NPG]          # [1200, 128]
        xT = np.ascontiguousarray(xd.T).reshape(N_FEAT, GPD, NPG).astype(npmm)
        bd = blocks[d * GPD:(d + 1) * GPD]                  # [8, 150, 150]
        bt = np.ascontiguousarray(bd.transpose(1, 0, 2))    # [150, 8, 150]
        in_maps.append(dict(
            xT=xT,
            ah0=np.ascontiguousarray(bt[:C0]),
            ah1=np.ascontiguousarray(bt[C0:]),
            **common,
        ))
    return in_maps


def kernel(x, a, seg_ids, num_graphs, W1, b1, W2, b2, Wa, ba, Wp, bp, Wc, bc,
           trace=False):
    if "nc" not in _CACHE:
        _CACHE["nc"] = build_nc()
    nc = _CACHE["nc"]
    in_maps = make_in_maps(x, a, W1, b1, W2, b2, Wa, ba, Wp, bp, Wc, bc)
    res = run_bass_kernel_spmd(nc, in_maps, core_ids=list(range(DEV)), trace=trace)
    logits = np.concatenate([r["out"] for r in res.results], axis=0)
    if trace:
        return logits, res
    return logits



# revision 32
# speedup vs baseline: 1.0465x; 1.0465x over previous
"""GCN + DiffPool kernel for Trainium2, data-parallel over graphs across 8 NeuronCores.

Model (per graph, n=150 nodes):
  Z1 = relu(An @ (x @ W1) + b1)          An = D^-1/2 (A+I) D^-1/2
  Z2 = relu(An @ (Z1 @ W2) + b2)
  S  = softmax(An @ (Z2 @ Wa) + ba)      [n, 25]
  Zp = S^T @ Z2 ; Ap = S^T @ (A @ S)
  H  = relu(Anp @ (Zp @ Wp) + bp)        pooled GCN, 25 cluster-nodes
  logits = (sum_rows H) @ Wc + bc

Sharding: 64 graphs -> 8 devices x 8 graphs (block-diagonal adjacency).

v3 structure (all normalization host-precomputed):
  - Host ships an = (A+I) .* d_col (column-normalized bf16) only; the raw
    adjacency is never shipped: A@S is recovered as dinv .* (an-contraction)
    with dinv = sqrt(deg) host-shipped. x is pre-scaled by d rows, so the
    m1 drain is a plain copy. Deferred normalization elsewhere: the row
    factor is folded into the moving operand m = d.*(Z@W) at drain time.
  - v and Y = Z2@Wp share their stationary (z2 slices): rhs = [Wa | Wp].
  - Fused pooling matmul: out[25, g, 90] = S^T @ [Y | AS | deg-1] gives
    Zp@Wp (cluster-major), Ap, and pooled degrees (rowsum(Ap) = S^T(deg-1))
    in one 16-matmul set; dp = rsqrt(1 + col89) via quake+Newton on DVE.
  - Pooled GCN computed cluster-major: H_cm = relu(dp .* ((dp-rows.*(Ap+I))
    @ ZpWp) + bpT) with dp on partitions both times -> no dp transpose /
    partition-broadcast needed; readout G = H_cm^T @ ones via tiny matmuls;
    logits as a single bf16 matmul.

Layouts: fm [feat_part, graph, node] for W-multiplies (lhsT), nm
[node_part, graph, feat] for A-multiplies. Node chunks c0=[0:128], c1=[128:150].
"""

import numpy as np

import concourse.bass as bass
import concourse.mybir as mybir
import concourse.tile as tile
from concourse import bacc
from concourse.bass_utils import run_bass_kernel_spmd

F32 = mybir.dt.float32
BF16 = mybir.dt.bfloat16
AF = mybir.ActivationFunctionType
AL = mybir.AluOpType
U32 = mybir.dt.uint32

MMDT = BF16

N_NODES = 9600
N_FEAT = 128
HIDDEN = 64
CLUSTERS = 25
NUM_CLASSES = 10
B_GRAPHS = 64
NPG = 150            # nodes per graph
DEV = 8              # devices
GPD = 8              # graphs per device
HG = GPD // 2
C0, C1 = 128, 22     # node partition chunks (128 + 22 = 150)
VYC = CLUSTERS + HIDDEN          # 89: [Wa | Wp] fused free dim
CATC = HIDDEN + CLUSTERS + 1     # 90: [Y | AS | degm1] fused free dim

_CACHE = {}

# wpk (bf16) packed-constant column offsets
WP_W1 = 0                       # [128, 64]
WP_W2 = WP_W1 + HIDDEN          # [64, 64]
WP_WAP = WP_W2 + HIDDEN         # [64, 89] = [Wa | Wp] fused
WP_WC = WP_WAP + VYC            # [64, 10]
WP_DM0 = WP_WC + NUM_CLASSES    # [128, 8] deg-1 chunk0, node-major
WP_DM1 = WP_DM0 + GPD           # [22, 8] deg-1 chunk1
WP_ONE = WP_DM1 + GPD           # [25, 1] ones column (readout contraction)
WP_COLS = WP_ONE + 1

# fpk (f32) packed-constant column offsets
FP_BC = 0                       # [8, 10] bc broadcast over graphs
FP_B1 = FP_BC + NUM_CLASSES     # [64, 1]
FP_B2 = FP_B1 + 1
FP_BA = FP_B2 + 1               # [128, 25] ba broadcast over partitions
FP_ID25 = FP_BA + CLUSTERS      # [25, 25] identity (ahp build)
FP_D0 = FP_ID25 + CLUSTERS      # [128, 8] d chunk0, node-major
FP_D1 = FP_D0 + GPD             # [22, 8] d chunk1
FP_DI0 = FP_D1 + GPD            # [128, 8] 1/d = sqrt(deg) chunk0
FP_DI1 = FP_DI0 + GPD           # [22, 8] chunk1
FP_BPT = FP_DI1 + GPD           # [25, 64] bp replicated over 25 partitions
FP_COLS = FP_BPT + HIDDEN


def _chunk(c):
    return (0, C0) if c == 0 else (C0, C1)


def build_nc():
    nc = bacc.Bacc("TRN2", target_bir_lowering=False, debug=False, num_devices=DEV)

    def din(name, shape, dt=F32):
        return nc.dram_tensor(name, shape, dt, kind="ExternalInput").ap()

    wpk = din("wpk", [N_FEAT, WP_COLS], MMDT)
    xT0 = din("xT0", [N_FEAT, HG, NPG], MMDT)    # d .* x, fm, graphs 0:4
    xT1 = din("xT1", [N_FEAT, HG, NPG], MMDT)    # graphs 4:8
    an00 = din("an00", [C0, HG, NPG], MMDT)      # (A+I).*d_col rows 0:128
    an01 = din("an01", [C0, HG, NPG], MMDT)
    an1 = din("an1", [C1 + 1, GPD, NPG], MMDT)   # rows 128:150 + ones row 22
    fpk = din("fpk", [N_FEAT, FP_COLS], F32)
    out = nc.dram_tensor("out", [GPD, NUM_CLASSES], F32, kind="ExternalOutput").ap()

    with tile.TileContext(nc) as tc:
        with (
            tc.tile_pool(name="cst", bufs=1) as cst,
            tc.tile_pool(name="act", bufs=1) as act,
            tc.tile_pool(name="ps", bufs=7, space="PSUM") as ps,
        ):
            # ---- input DMAs; wpk + xT0 head the m1 chain ------------------
            s_wpk = cst.tile([N_FEAT, WP_COLS], MMDT, tag="wpk")
            nc.sync.dma_start(out=s_wpk[:], in_=wpk)
            s_an1 = cst.tile([C1 + 1, GPD, NPG], MMDT, tag="an1")
            nc.scalar.dma_start(out=s_an1[:], in_=an1)
            s_xT = cst.tile([N_FEAT, GPD, NPG], MMDT, tag="xT")
            nc.sync.dma_start(out=s_xT[:, 0:HG, :], in_=xT0)
            s_fpk = cst.tile([N_FEAT, FP_COLS], F32, tag="fpk")
            nc.scalar.dma_start(out=s_fpk[:], in_=fpk)
            s_an0 = cst.tile([C0, GPD, NPG], MMDT, tag="an0")
            nc.sync.dma_start(out=s_an0[:, 0:HG, :], in_=an00)
            nc.scalar.dma_start(out=s_xT[:, HG:GPD, :], in_=xT1)
            nc.sync.dma_start(out=s_an0[:, HG:GPD, :], in_=an01)

            s_W1 = s_wpk[:, WP_W1:WP_W1 + HIDDEN]
            s_W2 = s_wpk[0:HIDDEN, WP_W2:WP_W2 + HIDDEN]
            s_WaP = s_wpk[0:HIDDEN, WP_WAP:WP_WAP + VYC]
            s_Wc = s_wpk[0:HIDDEN, WP_WC:WP_WC + NUM_CLASSES]
            s_dm = [s_wpk[:, WP_DM0:WP_DM0 + GPD],
                    s_wpk[0:C1, WP_DM1:WP_DM1 + GPD]]
            s_one25 = s_wpk[0:CLUSTERS, WP_ONE:WP_ONE + 1]
            s_bc = s_fpk[0:GPD, FP_BC:FP_BC + NUM_CLASSES]
            s_b1 = s_fpk[0:HIDDEN, FP_B1:FP_B1 + 1]
            s_b2 = s_fpk[0:HIDDEN, FP_B2:FP_B2 + 1]
            s_baB = s_fpk[:, FP_BA:FP_BA + CLUSTERS]
            s_id25 = s_fpk[0:CLUSTERS, FP_ID25:FP_ID25 + CLUSTERS]
            s_d = [s_fpk[:, FP_D0:FP_D0 + GPD],
                   s_fpk[0:C1, FP_D1:FP_D1 + GPD]]
            s_di = [s_fpk[:, FP_DI0:FP_DI0 + GPD],
                    s_fpk[0:C1, FP_DI1:FP_DI1 + GPD]]
            s_bpT = s_fpk[0:CLUSTERS, FP_BPT:FP_BPT + HIDDEN]

            # quake rsqrt constants (dp only)
            qk1 = act.tile([CLUSTERS, 1], U32, tag="qk1")
            nc.vector.memset(qk1[:], 1)
            qkm = act.tile([CLUSTERS, 1], U32, tag="qkm")
            nc.vector.memset(qkm[:], 0x5F3759DF)

            # rhscat: [Y | AS | degm1] node-major; degm1 lands first
            rhscat = []
            for c, cn in ((0, C0), (1, C1)):
                t = act.tile([cn, GPD, CATC], MMDT, tag=f"rhscat{c}")
                nc.vector.tensor_copy(t[:, :, CATC - 1:CATC],
                                      s_dm[c][:][:, :, None])
                rhscat.append(t)

            # ---- helpers ---------------------------------------------------
            def w_mult_nm(lhs_fm, w, kdim, fout, name, dscale=True):
                """m = [d .*] (Z @ W), node-major chunks. lhsT = fm slice."""
                outs = []
                for c, cn in ((0, C0), (1, C1)):
                    off, _ = _chunk(c)
                    p = ps.tile([cn, GPD, fout], F32, tag="ps")
                    for g in range(GPD):
                        nc.tensor.matmul(p[:, g, :],
                                         lhs_fm[0:kdim, g, off:off + cn],
                                         w, start=True, stop=True)
                    o = act.tile([cn, GPD, fout], MMDT, tag=f"{name}{c}")
                    if dscale:
                        dbc = s_d[c][:][:, :, None].broadcast_to((cn, GPD, fout))
                        nc.vector.tensor_mul(o[:], p[:], dbc)
                    else:
                        nc.scalar.copy(o[:], p[:])
                    outs.append(o)
                return outs

            def an_mult_fm(m_nm, bias, name):
                """fm out [64, g, 150] = relu(An @ m + bias), graph-paired
                PSUM tiles, ACT-only drain."""
                o = act.tile([HIDDEN, GPD, NPG], MMDT, tag=name)
                for q in range(GPD // 2):
                    p = ps.tile([HIDDEN, 2, NPG], F32, tag="ps")
                    for k in range(2):
                        g = 2 * q + k
                        nc.tensor.matmul(p[:, k, :], m_nm[0][:, g, :],
                                         s_an0[:, g, :], start=True, stop=False)
                        nc.tensor.matmul(p[:, k, :], m_nm[1][0:C1, g, :],
                                         s_an1[0:C1, g, :], start=False, stop=True)
                    nc.scalar.activation(o[:, 2 * q:2 * q + 2, :], p[:],
                                         AF.Relu, bias=bias)
                return o

            # ---- encoder ---------------------------------------------------
            m1 = w_mult_nm(s_xT, s_W1, N_FEAT, HIDDEN, "m1", dscale=False)
            z1 = an_mult_fm(m1, s_b1, "z1")
            m2 = w_mult_nm(z1, s_W2, HIDDEN, HIDDEN, "m2")
            z2 = an_mult_fm(m2, s_b2, "z2")

            # ---- fused v | Y: rhs = [Wa | Wp], shared stationary z2 --------
            v = []
            for c, cn in ((0, C0), (1, C1)):
                off, _ = _chunk(c)
                rows = cn + (1 if c == 1 else 0)
                o = act.tile([rows, GPD, CLUSTERS], MMDT, tag=f"v{c}")
                if c == 1:
                    bsrc = s_baB[0:rows, 0:CLUSTERS][:, None, :] \
                        .broadcast_to((rows, GPD, CLUSTERS))
                    nc.vector.tensor_copy(o[:], bsrc)
                for h in range(2):
                    gl, gh = h * HG, (h + 1) * HG
                    p = ps.tile([cn, HG, 128], F32, tag="ps")
                    for k in range(HG):
                        nc.tensor.matmul(p[:, k, 0:VYC],
                                         z2[0:HIDDEN, gl + k, off:off + cn],
                                         s_WaP, start=True, stop=True)
                    dbc = s_d[c][:, gl:gh][:, :, None] \
                        .broadcast_to((cn, HG, CLUSTERS))
                    nc.vector.tensor_mul(o[0:cn, gl:gh, :],
                                         p[:, :, 0:CLUSTERS], dbc)
                    nc.scalar.copy(rhscat[c][:, gl:gh, 0:HIDDEN],
                                   p[:, :, CLUSTERS:VYC])
                v.append(o)

            # ---- S = softmax(An @ v + ba): ba rides an1 ones-row ----------
            s_P, s_S = [], []
            for mc, mn in ((0, C0), (1, C1)):
                moff, _ = _chunk(mc)
                p = ps.tile([mn, GPD, CLUSTERS], F32, tag="ps")
                for g in range(GPD):
                    nc.tensor.matmul(p[:, g, :], s_an0[:, g, moff:moff + mn],
                                     v[0][:, g, :], start=True, stop=False)
                    nc.tensor.matmul(p[:, g, :], s_an1[0:C1 + 1, g, moff:moff + mn],
                                     v[1][0:C1 + 1, g, :], start=False, stop=True)
                s_P.append(p)
                st = act.tile([mn, GPD, CLUSTERS], MMDT, tag=f"s{mc}")
                s_S.append(st)
            for h in range(2):
                gl, gh = h * HG, (h + 1) * HG
                for mc, mn in ((0, C0), (1, C1)):
                    p, s = s_P[mc], s_S[mc]
                    e = act.tile([mn, HG, CLUSTERS], F32, tag=f"e{mc}{h}")
                    nc.scalar.activation(e[:], p[:, gl:gh, :], AF.Exp)
                    ssum = act.tile([mn, HG], F32, tag=f"ssum{mc}{h}")
                    nc.vector.reduce_sum(out=ssum[:], in_=e[:],
                                         axis=mybir.AxisListType.X)
                    rs = act.tile([mn, HG], F32, tag=f"rs{mc}{h}")
                    nc.vector.reciprocal(rs[:], ssum[:])
                    nc.vector.tensor_mul(
                        s[:, gl:gh, :], e[:],
                        rs[:][:, :, None].broadcast_to((mn, HG, CLUSTERS)))

            # ---- AS = A @ S = dinv .* (an-contraction) - S ----------------
            for mc, mn in ((0, C0), (1, C1)):
                moff, _ = _chunk(mc)
                p = ps.tile([mn, GPD, CLUSTERS], F32, tag="ps")
                for g in range(GPD):
                    nc.tensor.matmul(p[:, g, :], s_an0[:, g, moff:moff + mn],
                                     s_S[0][:, g, :], start=True, stop=False)
                    nc.tensor.matmul(p[:, g, :], s_an1[0:C1, g, moff:moff + mn],
                                     s_S[1][:, g, :], start=False, stop=True)
                for h in range(2):
                    gl, gh = h * HG, (h + 1) * HG
                    dib = s_di[mc][:, gl:gh][:, :, None] \
                        .broadcast_to((mn, HG, CLUSTERS))
                    nc.vector.tensor_mul(
                        rhscat[mc][:, gl:gh, HIDDEN:HIDDEN + CLUSTERS],
                        p[:, gl:gh, :], dib)
                    nc.vector.tensor_tensor(
                        rhscat[mc][:, gl:gh, HIDDEN:HIDDEN + CLUSTERS],
                        rhscat[mc][:, gl:gh, HIDDEN:HIDDEN + CLUSTERS],
                        s_S[mc][:, gl:gh, :], AL.subtract)

            # ---- fused pooling matmul + cluster-major pooled GCN, per half
            s_G = act.tile([HIDDEN, GPD, 1], MMDT, tag="g")
            for h in range(2):
                gl = h * HG
                p_cat = ps.tile([CLUSTERS, HG, 128], F32, tag="ps")
                for k in range(HG):
                    g = gl + k
                    nc.tensor.matmul(p_cat[:, k, 0:CATC], s_S[0][:, g, :],
                                     rhscat[0][:, g, :], start=True, stop=False)
                    nc.tensor.matmul(p_cat[:, k, 0:CATC], s_S[1][:, g, :],
                                     rhscat[1][:, g, :], start=False, stop=True)
                # mpraw (no dp scale; dp enters via anp rows + final scale)
                mpraw = act.tile([CLUSTERS, HG, HIDDEN], MMDT, tag=f"mp{h}")
                nc.scalar.copy(mpraw[:], p_cat[:, :, 0:HIDDEN])
                # dp = rsqrt(1 + degp), quake + 1 Newton
                degp = act.tile([CLUSTERS, HG], F32, tag=f"degp{h}")
                nc.vector.tensor_scalar_add(degp[:],
                                            p_cat[:, :, CATC - 1:CATC], 1.0)
                dpw = act.tile([CLUSTERS, HG], F32, tag=f"dpw{h}")
                nc.vector.tensor_tensor(dpw[:].bitcast(U32),
                                        degp[:].bitcast(U32),
                                        qk1[:].broadcast_to((CLUSTERS, HG)),
                                        AL.logical_shift_right)
                nc.vector.tensor_tensor(dpw[:].bitcast(U32),
                                        qkm[:].broadcast_to((CLUSTERS, HG)),
                                        dpw[:].bitcast(U32), AL.subtract)
                dpw2 = act.tile([CLUSTERS, HG], F32, tag=f"dpw2{h}")
                nc.vector.tensor_mul(dpw2[:], dpw[:], dpw[:])
                nc.vector.tensor_mul(dpw2[:], dpw2[:], degp[:])
                nc.vector.tensor_scalar(dpw2[:], dpw2[:], -0.5, 1.5,
                                        AL.mult, AL.add)
                dp = act.tile([CLUSTERS, HG], F32, tag=f"dp{h}")
                nc.vector.tensor_mul(dp[:], dpw[:], dpw2[:])
                # anp = dp-rows .* (Ap + I)
                anps = act.tile([CLUSTERS, HG, CLUSTERS], F32, tag=f"anps{h}")
                nc.vector.tensor_add(
                    anps[:], p_cat[:, :, HIDDEN:HIDDEN + CLUSTERS],
                    s_id25[:, None, :].broadcast_to((CLUSTERS, HG, CLUSTERS)))
                anp = act.tile([CLUSTERS, HG, CLUSTERS], MMDT, tag=f"anp{h}")
                nc.vector.tensor_mul(
                    anp[:], anps[:],
                    dp[:][:, :, None].broadcast_to((CLUSTERS, HG, CLUSTERS)))
                # H_cm = relu(dp .* (anp^T @ mpraw) + bpT)
                p_hc = ps.tile([CLUSTERS, HG, HIDDEN], F32, tag="ps")
                for k in range(HG):
                    nc.tensor.matmul(p_hc[:, k, :], anp[:, k, :],
                                     mpraw[:, k, :], start=True, stop=True)
                t1 = act.tile([CLUSTERS, HG, HIDDEN], F32, tag=f"t1{h}")
                nc.vector.tensor_mul(
                    t1[:], p_hc[:],
                    dp[:][:, :, None].broadcast_to((CLUSTERS, HG, HIDDEN)))
                nc.vector.tensor_add(
                    t1[:], t1[:],
                    s_bpT[:][:, None, :].broadcast_to((CLUSTERS, HG, HIDDEN)))
                hcm = act.tile([CLUSTERS, HG, HIDDEN], MMDT, tag=f"hcm{h}")
                nc.scalar.activation(hcm[:], t1[:], AF.Relu)
                # G = H_cm^T @ ones  (readout over clusters)
                p_g = ps.tile([HIDDEN, HG, 1], F32, tag="ps")
                for k in range(HG):
                    nc.tensor.matmul(p_g[:, k, :], hcm[:, k, :], s_one25,
                                     start=True, stop=True)
                with nc.allow_low_precision(reason="bf16 G for 1-mm logits"):
                    nc.vector.tensor_copy(s_G[:, gl:gl + HG, :], p_g[:])

            p_l = ps.tile([GPD, NUM_CLASSES], F32, tag="ps")
            nc.tensor.matmul(p_l[:], s_G[:], s_Wc, start=True, stop=True)
            s_out = act.tile([GPD, NUM_CLASSES], F32, tag="logits")
            nc.vector.tensor_add(s_out[:], p_l[:], s_bc)
            nc.sync.dma_start(out=out, in_=s_out[:])

    nc.compile()
    return nc


def make_in_maps(x, a, W1, b1, W2, b2, Wa, ba, Wp, bp, Wc, bc):
    import ml_dtypes
    npmm = np.dtype(ml_dtypes.bfloat16) if MMDT == BF16 else np.dtype(np.float32)

    x = np.ascontiguousarray(np.asarray(x, dtype=np.float32))
    a = np.asarray(a, dtype=np.float32)

    ab = a.reshape(B_GRAPHS, NPG, B_GRAPHS, NPG)
    blocks = ab[np.arange(B_GRAPHS), :, np.arange(B_GRAPHS), :]
    blocks = blocks + np.eye(NPG, dtype=np.float32)[None]

    deg = blocks.sum(axis=2)                       # [64, 150]
    d = 1.0 / np.sqrt(np.maximum(deg, 1e-12))
    dinv = np.sqrt(np.maximum(deg, 1e-12))
    degm1 = (deg - 1.0)

    an_blocks = (blocks * d[:, None, :]).astype(npmm)   # column-normalized
    dflat = d.reshape(-1)
    xs = x * dflat[:, None]

    wpk = np.zeros((N_FEAT, WP_COLS), npmm)
    wpk[:, WP_W1:WP_W1 + HIDDEN] = np.asarray(W1, np.float32).astype(npmm)
    wpk[0:HIDDEN, WP_W2:WP_W2 + HIDDEN] = np.asarray(W2, np.float32).astype(npmm)
    wpk[0:HIDDEN, WP_WAP:WP_WAP + CLUSTERS] = np.asarray(Wa, np.float32).astype(npmm)
    wpk[0:HIDDEN, WP_WAP + CLUSTERS:WP_WAP + VYC] = \
        np.asarray(Wp, np.float32).astype(npmm)
    wpk[0:HIDDEN, WP_WC:WP_WC + NUM_CLASSES] = np.asarray(Wc, np.float32).astype(npmm)
    wpk[0:CLUSTERS, WP_ONE] = 1.0

    fpk = np.zeros((N_FEAT, FP_COLS), np.float32)
    fpk[0:GPD, FP_BC:FP_BC + NUM_CLASSES] = np.asarray(bc, np.float32)[None, :]
    fpk[0:HIDDEN, FP_B1] = np.asarray(b1, np.float32)
    fpk[0:HIDDEN, FP_B2] = np.asarray(b2, np.float32)
    fpk[:, FP_BA:FP_BA + CLUSTERS] = np.asarray(ba, np.float32)[None, :]
    fpk[0:CLUSTERS, FP_ID25:FP_ID25 + CLUSTERS] = np.eye(CLUSTERS, dtype=np.float32)
    fpk[0:CLUSTERS, FP_BPT:FP_BPT + HIDDEN] = np.asarray(bp, np.float32)[None, :]

    in_maps = []
    for dev in range(DEV):
        gs = slice(dev * GPD, (dev + 1) * GPD)
        xd = xs[dev * GPD * NPG:(dev + 1) * GPD * NPG]
        xTd = np.ascontiguousarray(xd.T).reshape(N_FEAT, GPD, NPG).astype(npmm)

        anb = np.ascontiguousarray(an_blocks[gs].transpose(1, 0, 2))  # [150,8,150]
        an1t = np.ones((C1 + 1, GPD, NPG), npmm)
        an1t[0:C1] = anb[C0:]

        dd = d[gs]
        di = dinv[gs]
        dm = degm1[gs]
        fpkd = fpk.copy()
        fpkd[0:C0, FP_D0:FP_D0 + GPD] = dd[:, 0:C0].T
        fpkd[0:C1, FP_D1:FP_D1 + GPD] = dd[:, C0:].T
        fpkd[0:C0, FP_DI0:FP_DI0 + GPD] = di[:, 0:C0].T
        fpkd[0:C1, FP_DI1:FP_DI1 + GPD] = di[:, C0:].T
        wpkd = wpk.copy()
        wpkd[0:C0, WP_DM0:WP_DM0 + GPD] = dm[:, 0:C0].T.astype(npmm)
        wpkd[0:C1, WP_DM1:WP_DM1 + GPD] = dm[:, C0:].T.astype(npmm)

        in_maps.append(dict(
            wpk=wpkd,
            xT0=np.ascontiguousarray(xTd[:, 0:HG, :]),
            xT1=np.ascontiguousarray(xTd[:, HG:GPD, :]),
            an00=np.ascontiguousarray(anb[:C0, 0:HG, :]),
            an01=np.ascontiguousarray(anb[:C0, HG:GPD, :]),
            an1=an1t,
            fpk=fpkd,
        ))
    return in_maps


def kernel(x, a, seg_ids, num_graphs, W1, b1, W2, b2, Wa, ba, Wp, bp, Wc, bc,
           trace=False):
    if "nc" not in _CACHE:
        _CACHE["nc"] = build_nc()
    nc = _CACHE["nc"]
    in_maps = make_in_maps(x, a, W1, b1, W2, b2, Wa, ba, Wp, bp, Wc, bc)
    res = run_bass_kernel_spmd(nc, in_maps, core_ids=list(range(DEV)), trace=trace)
    logits = np.concatenate([r["out"] for r in res.results], axis=0)
    if trace:
        return logits, res
    return logits
